# revision 40
# baseline (speedup 1.0000x reference)
"""AdMSoftmaxLoss distributed Trainium2 kernel.

Reference computation (N=8192, D=1024, C=10240, S=30, ml=0.4, ms=0.1):
    wf    = clip(l2norm(x) @ l2norm(weight).T, -1, 1)      # (N, C) cosines
    m     = where(labels <= 5, ml, ms)
    t     = wf[i, labels[i]]
    num   = S * (t - m)
    excl  = sum_j exp(S * wf[i, j]) - exp(S * t)
    L     = num - log(exp(num) + excl)
    loss  = -mean(L)

Sharding: 2 row-groups x 4 class-groups over 8 NeuronCores. Core i gets
rows [(i//4)*4096, ..) and classes [(i%4)*2560, ..).

Work split:
  - Host (O((N+C)*D), ~0.01% of total FLOPs): l2-normalize x and weight,
    scale by 64, quantize to fp8e4m3 (clipped to +-240 so the OCP grid
    matches TRN fp8e4), transpose to d-major, and gather the label target
    t[i] = xn[i] . wn[labels[i]] with one einsum.
  - Device (Theta(N*C*D) matmul + Theta(N*C) exp/row-sum): per core,
    cos*4096 = x8^T . w8 via fp8 DoubleRow matmuls (K=256 per
    instruction), ScalarE exp(psum * 30/4096) -> bf16, VectorE row-sum.
    Output: out[p, m] = sum_{c in shard} exp(S*cos[row, c]), row=m*128+p.
  - Host epilogue: total = sum of 4 class-shard partials, then the O(N)
    loss arithmetic in f64.

Schedule notes (measured on hw via perfetto/NTFF traces):
  - 640 DoubleRow matmuls (N=512) run back-to-back at 216ns each with
    LDWEIGHTS fully hidden — the fp8 silicon floor (~138us).
  - ~7us fixed NEFF preamble; first DMA data ~9us. 55 dummy warm-up
    matmuls fill that window so the PE HAM clock gate is at 8/8 when
    the real stream starts.
  - DMA rings are FIFO internally but fair-share the 16 engines, so
    only soon-needed transfers are queued up front: sync ring carries
    w0, the densely-packed m-tile-0 head of x, then w1..w4; the scalar
    ring carries the rest of x0 and x1; x2..x7 are issued from inside
    the ScalarE ACT stream (data-dependent staging). Whole 512-column
    chunks only — column-split transfers drop ring efficiency ~35%.
  - g0+g1 are processed chunk-outer (8 m-tiles per chunk-row) so w_n is
    first needed at ~n*6.8us; from g2 on, m-tile-outer with chunk pairs
    in [128, 1024] psum tiles (4 bufs = 8 banks).
  - Epilogue per psum tile: ScalarE exp -> bf16 esc, DVE in-place
    tensor_scalar with accum_out for the row-sum; the last m-tile uses
    ACT accum_out instead so the final chain skips the DVE queue.
"""

import os
import sys
import types

import numpy as np


def _ensure_ntff_hook():
    """Make bass_utils' trace=True path usable: some containers ship an
    antenv stub without axon_hooks, which crashes run_bass_kernel_spmd
    when tracing is requested. Install the ctypes-based hook shim."""
    try:
        import antenv.axon_hooks  # noqa: F401

        return
    except ImportError:
        pass
    try:
        from trn_agent_boot.trn_boot import _ntff_profile_via_ctypes

        hook = _ntff_profile_via_ctypes("/opt/axon/libaxon_pjrt.so")
    except Exception:
        hook = None
    mod = types.ModuleType("antenv.axon_hooks")
    mod.get_axon_ntff_profile_hook = lambda: hook
    mod.set_axon_ntff_profile_hook = lambda h: None
    sys.modules["antenv.axon_hooks"] = mod
    try:
        import antenv

        antenv.axon_hooks = mod
    except ImportError:
        pass


P = 128
N_ROWS, D, C = 8192, 1024, 10240
S = 30.0
ML, MS = 0.4, 0.1
NCORES = 8
RG, CG = 2, 4                  # row groups x class groups
R_LOC = N_ROWS // RG           # 4096
C_LOC = C // CG                # 2560
M_TILES = R_LOC // P           # 32
K_TILES = D // P               # 8
NCHUNK = 512
N_CHUNKS = C_LOC // NCHUNK     # 5
X_CHUNKS = R_LOC // NCHUNK     # 8
JPC = NCHUNK // P              # 4 m-tiles per x chunk
N_WARM = 55                    # PE warm-up matmuls (~5us at cold clock)

QSCALE = 64.0                  # fp8 quantization scale for xn and wn
SEXP = S / (QSCALE * QSCALE)   # exp scale: psum = QSCALE^2 * cos

_CACHE = {}
LAST_RESULTS = None  # BassKernelResults of the most recent run (for test.py)


def _build():
    """Build + compile the SPMD Bass graph once; cache in module global."""
    if "nc" in _CACHE:
        return _CACHE["nc"]

    import concourse.bass as bass
    import concourse.mybir as mybir
    import concourse.tile as tile
    from concourse import bacc

    dt = mybir.dt
    AF = mybir.ActivationFunctionType
    ALU = mybir.AluOpType

    nc = bacc.Bacc(
        "TRN2", target_bir_lowering=False, debug=False, num_devices=NCORES
    )

    x_ext = nc.dram_tensor(
        "x8", [P, K_TILES, R_LOC], dt.float8e4, kind="ExternalInput"
    ).ap()
    xh_ext = nc.dram_tensor(
        "xh", [P, K_TILES, P], dt.float8e4, kind="ExternalInput"
    ).ap()
    w_ext = nc.dram_tensor(
        "w8", [P, K_TILES, C_LOC], dt.float8e4, kind="ExternalInput"
    ).ap()
    out_ext = nc.dram_tensor(
        "out", [P, M_TILES], dt.float32, kind="ExternalOutput"
    ).ap()

    with tile.TileContext(nc) as tc:
        with (
            tc.tile_pool(name="consts", bufs=1) as consts,
            tc.tile_pool(name="esc", bufs=4) as esc,
            tc.tile_pool(name="psum", bufs=4, space="PSUM") as psum,
        ):
            wsb = [
                consts.tile([P, K_TILES, NCHUNK], dt.float8e4, name=f"w{n}")
                for n in range(N_CHUNKS)
            ]
            xsb = [
                consts.tile([P, K_TILES, NCHUNK], dt.float8e4, name=f"x{g}")
                for g in range(X_CHUNKS)
            ]
            sums = consts.tile([P, M_TILES, N_CHUNKS], dt.float32)
            outsum = consts.tile([P, M_TILES], dt.float32)
            warm = consts.tile([P, P], dt.bfloat16)

            # Warm-up source + input DMAs. gpsimd memsets `warm` first so
            # the PE can run dummy matmuls while real data streams in.
            # Rings are FIFO internally but share the 16 DMA engines
            # roughly fairly (~145 GB/s each when two rings are active),
            # so only what's needed soon goes in up front: the sync ring
            # carries m-tile 0 of x plus all w chunks (the critical
            # startup path); the scalar ring carries the rest of x0-x2.
            # x3..x7 are issued from inside the scalar engine's ACT
            # stream so they can't crowd the w ring during startup.
            # Sync ring (FIFO): m-tile 0 of x, then whole w chunks —
            # full 512B-run transfers keep the ring at full rate (column
            # splits drop DMA efficiency ~35%). Scalar ring: rest of x0
            # and x1; x2..x7 are staged from the ACT stream.
            # m-tile 0 comes from its own densely-packed DRAM tensor
            # (contiguous 1KB runs — the strided [P,8,128] slice of x8
            # only sustains ~70GB/s).
            xhead = consts.tile([P, K_TILES, P], dt.float8e4, name="xh_sb")
            nc.gpsimd.memset(warm[:], 0.0)
            nc.sync.dma_start(wsb[0][:], w_ext[:, :, 0:NCHUNK])
            nc.sync.dma_start(xhead[:], xh_ext)
            for n in range(1, N_CHUNKS):
                nc.sync.dma_start(wsb[n][:], w_ext[:, :, bass.ts(n, NCHUNK)])
            nc.scalar.dma_start(xsb[0][:, :, P:NCHUNK], x_ext[:, :, P:NCHUNK])
            nc.scalar.dma_start(xsb[1][:], x_ext[:, :, bass.ts(1, NCHUNK)])

            # PE HAM warm-up: ~6us of back-to-back tiny matmuls so the
            # clock gate reaches 8/8 before the first real matmul.
            wp = psum.tile([P, 2 * NCHUNK], dt.float32, tag="ps")
            for _ in range(N_WARM):
                nc.tensor.matmul(
                    wp[:, 0:P], warm[:], warm[:], start=True, stop=True
                )

            def mm_chunk(g, n, j, ps_slice):
                """4 DoubleRow matmuls: m-tile j of x-chunk g vs chunk n."""
                head = g == 0 and j == 0
                for kp in range(K_TILES // 2):
                    lhs = (
                        xhead[:, 2 * kp : 2 * kp + 2, :]
                        if head
                        else xsb[g][:, 2 * kp : 2 * kp + 2, bass.ts(j, P)]
                    )
                    nc.tensor.matmul(
                        ps_slice,
                        lhs,
                        wsb[n][:, 2 * kp : 2 * kp + 2, :],
                        start=(kp == 0),
                        stop=(kp == K_TILES // 2 - 1),
                        perf_mode=mybir.MatmulPerfMode.DoubleRow,
                    )

            def epilogue(jg, slot, ps, width, on_act=False):
                """exp + row-sum of ps[:, :width] into sums[:, jg, slot].
                ACT exp -> bf16 esc; DVE tensor_scalar + accum. on_act
                folds the row-sum into the ACT itself (used for the last
                m-tile so the final chain skips the DVE queue)."""
                e = esc.tile([P, 2 * NCHUNK], dt.bfloat16, tag="esc")
                nc.scalar.activation(
                    e[:, :width],
                    ps[:, :width],
                    AF.Exp,
                    scale=SEXP,
                    accum_out=sums[:, jg, slot : slot + 1] if on_act else None,
                )
                if not on_act:
                    nc.vector.tensor_scalar(
                        e[:, :width],
                        e[:, :width],
                        1.0,
                        0.0,
                        ALU.mult,
                        ALU.add,
                        accum_out=sums[:, jg, slot : slot + 1],
                    )

            def do_tile(g, n, j):
                """Single-chunk tile (startup phase): half-used psum."""
                jg = g * JPC + j
                ps = psum.tile([P, 2 * NCHUNK], dt.float32, tag="ps")
                mm_chunk(g, n, j, ps[:, 0:NCHUNK])
                epilogue(jg, n, ps, NCHUNK)

            def do_pair(g, n0, j, slot, on_act=False):
                """Chunk pair (n0, n0+1) of m-tile j in one psum tile."""
                jg = g * JPC + j
                ps = psum.tile([P, 2 * NCHUNK], dt.float32, tag="ps")
                mm_chunk(g, n0, j, ps[:, 0:NCHUNK])
                mm_chunk(g, n0 + 1, j, ps[:, NCHUNK : 2 * NCHUNK])
                epilogue(jg, slot, ps, 2 * NCHUNK, on_act)

            def do_single(g, n, j, slot, on_act=False):
                jg = g * JPC + j
                ps = psum.tile([P, 2 * NCHUNK], dt.float32, tag="ps")
                mm_chunk(g, n, j, ps[:, 0:NCHUNK])
                epilogue(jg, slot, ps, NCHUNK, on_act)

            def finish_tile(jg, nslots):
                """Fold the chunk sums of m-tile jg into the output."""
                nc.vector.tensor_reduce(
                    outsum[:, jg : jg + 1],
                    sums[:, jg, 0:nslots],
                    axis=mybir.AxisListType.X,
                    op=ALU.add,
                )

            def stage_x(g):
                """Issue x-chunk g's load from the scalar stream (runs
                only when the preceding ACT has executed, keeping the
                fabric clear for w during startup)."""
                nc.scalar.dma_start(xsb[g][:], x_ext[:, :, bass.ts(g, NCHUNK)])

            # g0+g1 fused, chunk-outer: each chunk-row is 8 m-tiles
            # (~6.8us), so w_n is first needed at ~n*6.8us — the sync
            # ring keeps pace with margin.
            for n in range(N_CHUNKS):
                for jg in range(2 * JPC):
                    do_tile(jg // JPC, n, jg % JPC)
                if n == 1:
                    stage_x(2)
                elif n == 3:
                    stage_x(3)
            for jg in range(2 * JPC):
                finish_tile(jg, N_CHUNKS)
            nc.sync.dma_start(out_ext[:, 0 : 2 * JPC], outsum[:, 0 : 2 * JPC])

            # steady state: chunk pairs in full [128, 1024] psum tiles
            for g in range(2, X_CHUNKS):
                for j in range(JPC):
                    last = (g, j) == (X_CHUNKS - 1, JPC - 1)
                    do_pair(g, 0, j, 0, on_act=last)
                    do_pair(g, 2, j, 1, on_act=last)
                    do_single(g, 4, j, 2, on_act=last)
                    finish_tile(g * JPC + j, 3)
                    if j == 0 and 2 <= g <= 5:
                        stage_x(g + 2)
                nc.sync.dma_start(
                    out_ext[:, bass.ts(g, JPC)], outsum[:, bass.ts(g, JPC)]
                )

    nc.compile()
    _CACHE["nc"] = nc
    return nc


def _quant8(a):
    """f32 -> TRN fp8e4 grid (OCP e4m3fn clipped to +-240)."""
    import ml_dtypes

    return np.clip(a, -240.0, 240.0).astype(ml_dtypes.float8_e4m3fn)


def _make_in_maps(xn8, wn8):
    """Shard + transpose to [P, K_TILES, cols] d-major layouts."""
    in_maps = []
    for i in range(NCORES):
        gr, ci = divmod(i, CG)
        xs = xn8[gr * R_LOC : (gr + 1) * R_LOC]  # (R_LOC, D)
        ws = wn8[ci * C_LOC : (ci + 1) * C_LOC]  # (C_LOC, D)
        # [r, k*128+p] -> [p, k, r]
        xT = np.ascontiguousarray(
            xs.T.reshape(K_TILES, P, R_LOC).transpose(1, 0, 2)
        )
        wT = np.ascontiguousarray(
            ws.T.reshape(K_TILES, P, C_LOC).transpose(1, 0, 2)
        )
        xh = np.ascontiguousarray(xT[:, :, 0:P])
        in_maps.append({"x8": xT, "w8": wT, "xh": xh})
    return in_maps


def kernel(x, labels, weight):
    global LAST_RESULTS
    _ensure_ntff_hook()
    from concourse.bass_utils import run_bass_kernel_spmd

    x = np.asarray(x, dtype=np.float32)
    weight = np.asarray(weight, dtype=np.float32)
    labels = np.asarray(labels)

    # Host: normalize (eps matches F.normalize), quantize, target gather.
    xn = x / np.maximum(np.linalg.norm(x, axis=1, keepdims=True), 1e-12)
    wn = weight / np.maximum(
        np.linalg.norm(weight, axis=1, keepdims=True), 1e-12
    )
    t = np.clip(np.einsum("nd,nd->n", xn, wn[labels]), -1.0, 1.0)
    xn8 = _quant8(QSCALE * xn)
    wn8 = _quant8(QSCALE * wn)

    nc = _build()
    in_maps = _make_in_maps(xn8, wn8)
    trace = bool(int(os.environ.get("ADMS_TRACE", "0")))
    res = run_bass_kernel_spmd(nc, in_maps, list(range(NCORES)), trace=trace)
    LAST_RESULTS = res

    total = np.zeros(N_ROWS, np.float64)
    for i, r in enumerate(res.results):
        gr = i // CG
        o = np.asarray(r["out"], dtype=np.float64)  # [P, M_TILES]
        sl = slice(gr * R_LOC, (gr + 1) * R_LOC)
        total[sl] += o.T.reshape(R_LOC)  # row = m*P + p

    t = t.astype(np.float64)
    m = np.where(labels <= 5, ML, MS)
    num = S * (t - m)
    L = num - np.log(np.exp(num) + (total - np.exp(S * t)))
    return np.float32(-L.mean())
